# revision 1
# baseline (speedup 1.0000x reference)
"""CLIP text embedding lookup on 8 TRN2 NeuronCores.

out[1, 77, 768] = token_weight[input_ids] + position_weight[position_ids]

Strategy: sequence-parallel. 77 positions are padded to 80 and split 10 per
core. Each core indirect-DMA-gathers its 10 token rows from the full
replicated [49408, 768] table (one descriptor per row, one dest partition per
row), adds the core's 10 position-embedding rows (pre-sharded on the host —
position_ids is a static arange; a general fallback gathers them on the host
only if a caller passes a permuted position_ids), and writes its [10, 768]
output slice. The host concatenates the 8 slices and trims to 77 rows.
"""

import numpy as np

NCORES = 8
SEQ = 77
DIM = 768
VOCAB = 49408
MAX_POS = 77
ROWS = 10  # ceil(77 / 8)
PAD_SEQ = NCORES * ROWS  # 80

# test.py can flip TRACE; LAST_RESULTS stashes BassKernelResults for test.py.
TRACE = False
LAST_RESULTS = None

_compiled = None


def _build():
    import concourse.bacc as bacc
    import concourse.bass as bass
    import concourse.mybir as mybir
    import concourse.tile as tile

    nc = bacc.Bacc(
        "TRN2", target_bir_lowering=False, debug=False, num_devices=NCORES
    )
    idx = nc.dram_tensor("idx", [ROWS, 1], mybir.dt.int32, kind="ExternalInput").ap()
    table = nc.dram_tensor(
        "table", [VOCAB, DIM], mybir.dt.float32, kind="ExternalInput"
    ).ap()
    pos = nc.dram_tensor(
        "pos", [ROWS, DIM], mybir.dt.float32, kind="ExternalInput"
    ).ap()
    out = nc.dram_tensor(
        "out", [ROWS, DIM], mybir.dt.float32, kind="ExternalOutput"
    ).ap()

    with tile.TileContext(nc) as tc:
        with tc.tile_pool(name="sbuf", bufs=1) as pool:
            idx_t = pool.tile([ROWS, 1], mybir.dt.int32)
            tok_t = pool.tile([ROWS, DIM], mybir.dt.float32)
            # idx load rides the gather's own SWDGE queue (no cross-engine
            # hop before the gather). Position rows go straight to the DRAM
            # output on the SP HWDGE queue, overlapping the gather chain;
            # the gathered token rows are then accumulated into the output
            # by the SWDGE CCE (accum_op=add), which removes both the
            # vector-add and its cross-engine semaphore hop.
            nc.gpsimd.dma_start(out=idx_t[:], in_=idx[:])
            nc.sync.dma_start(out=out[:], in_=pos[:])
            nc.gpsimd.indirect_dma_start(
                out=tok_t[:],
                out_offset=None,
                in_=table[:],
                in_offset=bass.IndirectOffsetOnAxis(ap=idx_t[:, :1], axis=0),
            )
            nc.gpsimd.dma_start(
                out=out[:], in_=tok_t[:], accum_op=mybir.AluOpType.add
            )
    nc.compile()
    return nc


def kernel(**inputs) -> np.ndarray:
    global _compiled, LAST_RESULTS
    from concourse.bass_utils import run_bass_kernel_spmd

    input_ids = np.asarray(inputs["input_ids"]).astype(np.int32).reshape(-1)
    position_ids = np.asarray(inputs["position_ids"]).astype(np.int64).reshape(-1)
    token_weight = np.ascontiguousarray(
        np.asarray(inputs["token_weight"], dtype=np.float32)
    )
    position_weight = np.ascontiguousarray(
        np.asarray(inputs["position_weight"], dtype=np.float32)
    )

    if _compiled is None:
        _compiled = _build()
    nc = _compiled

    ids_pad = np.zeros(PAD_SEQ, np.int32)
    ids_pad[:SEQ] = input_ids
    # Shard the (replicated) position table by sequence position. For the
    # canonical arange position_ids this is a pure row-shard; any other
    # permutation is resolved host-side the same way.
    pos_rows = position_weight[position_ids]  # [SEQ, DIM]
    pos_pad = np.zeros((PAD_SEQ, DIM), np.float32)
    pos_pad[:SEQ] = pos_rows

    in_maps = []
    for c in range(NCORES):
        sl = slice(c * ROWS, (c + 1) * ROWS)
        in_maps.append(
            {
                "idx": ids_pad[sl].reshape(ROWS, 1),
                "table": token_weight,
                "pos": pos_pad[sl],
            }
        )

    res = run_bass_kernel_spmd(nc, in_maps, list(range(NCORES)), trace=TRACE)
    LAST_RESULTS = res
    out = np.concatenate([r["out"] for r in res.results], axis=0)[:SEQ]
    return out[None]



# revision 3
# speedup vs baseline: 1.3270x; 1.3270x over previous
"""CLIP text embedding lookup on 8 TRN2 NeuronCores.

out[1, 77, 768] = token_weight[input_ids] + position_weight[position_ids]

Strategy: sequence-parallel. 77 positions are padded to 80 and split 10 per
core. Each core gathers its 10 token rows from the full replicated
[49408, 768] table, accumulates them onto its position rows in DRAM, and
writes its [10, 768] output slice. The host concatenates the 8 slices and
trims to 77 rows.

Each 768-float row is handled as 6 chunks of 128 floats (512 B): the host
expands every row index i into 6 chunk indices 6*i + c over the table viewed
as [49408*6, 128], and the gather lands the 60 chunks in a [60, 128] SBUF
tile (one partition per chunk). This keeps every SBUF access pattern
partition-canonical while dropping the per-first-dim transfer size of every
DMA to 512 B, which puts their descriptor-generation engine time at the
500 ns floor (vs 1184 ns for a [10, 768]-shaped access pattern).

All data-dependent DMAs ride one SWDGE queue on the Pool engine, so ordering
is queue-FIFO and the critical path is descriptor-generation-bound — no
cross-queue semaphore chains:
  Pool:  idx60 -> SBUF;  gather 60 table chunks -> SBUF [60, 128];
         SBUF -> out with accum_op=add.
  SP:    pos rows -> out (HWDGE, overlaps the Pool chain; out is
         zero-initialized by the runtime so accumulate semantics hold).
"""

import numpy as np

NCORES = 8
SEQ = 77
DIM = 768
VOCAB = 49408
MAX_POS = 77
ROWS = 10  # ceil(77 / 8)
PAD_SEQ = NCORES * ROWS  # 80

CHUNK = 128
CHUNKS_PER_ROW = DIM // CHUNK  # 6
NCHUNK = ROWS * CHUNKS_PER_ROW  # 60

# test.py can flip TRACE; LAST_RESULTS stashes BassKernelResults for test.py.
TRACE = False
LAST_RESULTS = None

_compiled = None


def _build():
    import concourse.bacc as bacc
    import concourse.bass as bass
    import concourse.mybir as mybir
    import concourse.tile as tile

    nc = bacc.Bacc(
        "TRN2", target_bir_lowering=False, debug=False, num_devices=NCORES
    )
    idx = nc.dram_tensor("idx", [NCHUNK, 1], mybir.dt.int32, kind="ExternalInput").ap()
    table = nc.dram_tensor(
        "table", [VOCAB, DIM], mybir.dt.float32, kind="ExternalInput"
    ).ap()
    pos = nc.dram_tensor(
        "pos", [ROWS, DIM], mybir.dt.float32, kind="ExternalInput"
    ).ap()
    out = nc.dram_tensor(
        "out", [ROWS, DIM], mybir.dt.float32, kind="ExternalOutput"
    ).ap()
    # The token table viewed as [49408*6, 128]: chunk index 6*i + c addresses
    # row i, elements 128*c .. 128*(c+1).
    table_chunks = bass.AP(
        tensor=table.tensor, offset=0, ap=[[CHUNK, VOCAB * CHUNKS_PER_ROW], [1, CHUNK]]
    )

    with tile.TileContext(nc) as tc:
        with tc.tile_pool(name="sbuf", bufs=1) as pool:
            idx_t = pool.tile([NCHUNK, 1], mybir.dt.int32)
            tok_t = pool.tile([NCHUNK, CHUNK], mybir.dt.float32)
            nc.gpsimd.dma_start(out=idx_t[:], in_=idx[:])
            nc.sync.dma_start(out=out[:], in_=pos[:])
            nc.gpsimd.indirect_dma_start(
                out=tok_t[:],
                out_offset=None,
                in_=table_chunks,
                in_offset=bass.IndirectOffsetOnAxis(ap=idx_t[:, :1], axis=0),
            )
            nc.gpsimd.dma_start(
                out=out[:], in_=tok_t[:], accum_op=mybir.AluOpType.add
            )
    nc.compile()
    return nc


def kernel(**inputs) -> np.ndarray:
    global _compiled, LAST_RESULTS
    from concourse.bass_utils import run_bass_kernel_spmd

    input_ids = np.asarray(inputs["input_ids"]).astype(np.int32).reshape(-1)
    position_ids = np.asarray(inputs["position_ids"]).astype(np.int64).reshape(-1)
    token_weight = np.ascontiguousarray(
        np.asarray(inputs["token_weight"], dtype=np.float32)
    )
    position_weight = np.ascontiguousarray(
        np.asarray(inputs["position_weight"], dtype=np.float32)
    )

    if _compiled is None:
        _compiled = _build()
    nc = _compiled

    ids_pad = np.zeros(PAD_SEQ, np.int32)
    ids_pad[:SEQ] = input_ids
    # Expand row indices to per-chunk indices over the [49408*6, 128] view.
    chunk_ids = (
        CHUNKS_PER_ROW * ids_pad[:, None] + np.arange(CHUNKS_PER_ROW)[None, :]
    ).astype(np.int32)  # [PAD_SEQ, 6]
    # Shard the (replicated) position table by sequence position. For the
    # canonical arange position_ids this is a pure row-shard; any other
    # permutation is resolved host-side the same way.
    pos_rows = position_weight[position_ids]  # [SEQ, DIM]
    pos_pad = np.zeros((PAD_SEQ, DIM), np.float32)
    pos_pad[:SEQ] = pos_rows

    in_maps = []
    for c in range(NCORES):
        sl = slice(c * ROWS, (c + 1) * ROWS)
        in_maps.append(
            {
                "idx": chunk_ids[sl].reshape(NCHUNK, 1),
                "table": token_weight,
                "pos": pos_pad[sl],
            }
        )

    res = run_bass_kernel_spmd(nc, in_maps, list(range(NCORES)), trace=TRACE)
    LAST_RESULTS = res
    out = np.concatenate([r["out"] for r in res.results], axis=0)[:SEQ]
    return out[None]
